# revision 3
# baseline (speedup 1.0000x reference)
"""ChoiceAttention Trainium2 kernel v2.

vs v1 (448 us HW):
  - scores land in ONE tile ST[P, k=5, jb=2, a=5, i=256] (diagonal a==k slots
    memset to -30000 by gpsimd): softmax batches elementwise ops over all a
    and k (k via stride-0 broadcast), cutting DVE instrs/item from ~85 to ~26.
  - softmax stages pipelined per jb (sub/exp/z/recip/mul/ws) so ACT exp
    overlaps DVE work.
  - item 3 (the tail) computes scores+softmax per i-half: softmax half 1
    hides under out(1)+out(2), out(3).ib0 needs only half 0.
  - dedicated 1-bank psum pool for out groups + item-3 score groups: out
    matmuls no longer wait on scores psum rotation.
  - W DMA split per-kc, interleaved into xt(0) option DMAs; nat deferred.
"""

import numpy as np

B, S, H = 32, 256, 1024
NCORES = 8
BPC = B // NCORES
P = 128
HC = H // P   # 8
SC = S // P   # 2
NOPT = 5

# scores psum layout per k: columns = a ascending, skipping a==k
SEGS = {
    0: [(0, 1, 2), (2, 3, 2)],
    1: [(0, 0, 1), (1, 2, 1), (2, 3, 2)],
    2: [(0, 0, 2), (2, 3, 2)],
    3: [(0, 0, 2), (2, 2, 1), (3, 4, 1)],
    4: [(0, 0, 2), (2, 2, 2)],
}

NEG = -30000.0

_CACHE: dict = {}


def _build_bass(reps: int = 1, cfg: dict | None = None):
    cfg = dict(cfg or {})
    BCAST = cfg.get("bcast", True)
    from contextlib import ExitStack, nullcontext

    import concourse.mybir as mybir
    import concourse.tile as tile
    from concourse import bacc

    FP32 = mybir.dt.float32
    F16 = mybir.dt.float16
    AF = mybir.ActivationFunctionType

    nc = bacc.Bacc(debug=False)

    xt_d = [nc.dram_tensor(f"xt{i + 1}", (BPC, H, S), F16, kind="ExternalInput")
            for i in range(NOPT)]
    nat_d = [nc.dram_tensor(f"nat{i + 1}", (BPC, S, H), F16, kind="ExternalInput")
             for i in range(NOPT)]
    w_d = nc.dram_tensor("W", (H, H), F16, kind="ExternalInput")
    out_d = nc.dram_tensor("out", (BPC, S, H), F16, kind="ExternalOutput")

    with ExitStack() as ctx:
        tc = ctx.enter_context(tile.TileContext(nc))
        const = ctx.enter_context(tc.tile_pool(name="const", bufs=1))
        xtp = ctx.enter_context(tc.tile_pool(name="xt", bufs=2))
        natp = ctx.enter_context(tc.tile_pool(name="nat", bufs=2))
        qp = ctx.enter_context(tc.tile_pool(name="qq", bufs=1))
        stp = ctx.enter_context(tc.tile_pool(name="st", bufs=2))
        mzp = ctx.enter_context(tc.tile_pool(name="mz", bufs=2))
        rp = ctx.enter_context(tc.tile_pool(name="rr", bufs=1))
        wsp = ctx.enter_context(tc.tile_pool(name="ws", bufs=3))
        op_ = ctx.enter_context(tc.tile_pool(name="osb", bufs=2))
        ps_big = ctx.enter_context(tc.tile_pool(name="ps_big", bufs=2, space="PSUM"))
        ps_sm = ctx.enter_context(tc.tile_pool(name="ps_sm", bufs=1, space="PSUM"))
        ps_o = ctx.enter_context(tc.tile_pool(name="ps_o", bufs=3, space="PSUM"))

        w_sb = const.tile([P, HC, H], F16)
        wap = w_d.ap().rearrange("(kc p) m -> p kc m", p=P)

        loop_cm = tc.For_i(0, reps, 1) if reps > 1 else nullcontext()
        with loop_cm:
            xts = [None] * BPC
            nats = [None] * BPC
            qs = [None] * BPC

            def xt_dma(b, kk):
                nc.sync.dma_start(
                    out=xts[b][:, :, kk, :],
                    in_=xt_d[kk].ap()[b].rearrange("(hc p) s -> p hc s", p=P))

            def load_xt(b, first=False):
                xts[b] = xtp.tile([P, HC, NOPT, S], F16, tag="xt", name=f"xt_{b}")
                if first:
                    # interleave: q(0) pass 1 starts after opt0.h0, opt1.h0, W0
                    def wc(kc):
                        nc.scalar.dma_start(out=w_sb[:, kc, :], in_=wap[:, kc, :])

                    def xt_half(kk, h):
                        hs = slice(h * 4, (h + 1) * 4)
                        nc.sync.dma_start(
                            out=xts[b][:, hs, kk, :],
                            in_=xt_d[kk].ap()[b].rearrange(
                                "(hc p) s -> p hc s", p=P)[:, hs, :])
                    xt_half(0, 0)
                    xt_half(1, 0)
                    wc(0)
                    wc(1)
                    xt_half(0, 1)
                    xt_half(1, 1)
                    wc(2)
                    wc(3)
                    xt_dma(b, 2)
                    wc(4)
                    wc(5)
                    xt_dma(b, 3)
                    for kc in range(6, HC):
                        wc(kc)
                    xt_dma(b, 4)
                else:
                    for kk in range(NOPT):
                        xt_dma(b, kk)

            def load_nat(b):
                nats[b] = natp.tile([P, NOPT, SC, H], F16, tag="nat",
                                    name=f"nat_{b}")
                for kk in range(NOPT):
                    nc.sync.dma_start(
                        out=nats[b][:, kk, :, :],
                        in_=nat_d[kk].ap()[b].rearrange("(sc p) h -> p sc h", p=P))

            def evac(out_ap, in_ap, eng="act", scale=None):
                # HW probe: psum->sbuf copies are cheap on ACT (~195ns for
                # [P,1024]f32) while DVE time is precious (softmax chain)
                if scale is None:
                    if eng == "act":
                        nc.scalar.copy(out=out_ap, in_=in_ap)
                    else:
                        nc.vector.tensor_copy(out=out_ap, in_=in_ap)
                else:
                    if eng == "act":
                        nc.scalar.activation(out=out_ap, in_=in_ap, func=AF.Copy,
                                             scale=scale)
                    else:
                        nc.vector.tensor_scalar_mul(out_ap, in_ap, scale)

            def emit_q(b):
                """Q(b) = q^T for all 5 options, W stationary, a-batched."""
                q = qp.tile([P, HC, NOPT, S], F16, tag="qq", name=f"q_{b}")
                for mc in range(HC):
                    qA = ps_big.tile([P, 4, S], FP32, tag="big", name=f"qA_{b}_{mc}")
                    qB = ps_sm.tile([P, S], FP32, tag="sm", name=f"qB_{b}_{mc}")
                    for kc in range(HC):
                        lhsT = w_sb[:, kc, mc * P:(mc + 1) * P]
                        st_, sp_ = (kc == 0), (kc == HC - 1)
                        nc.tensor.matmul(qA[:, 0:2, :], lhsT, xts[b][:, kc, 0:2, :],
                                         start=st_, stop=sp_)
                        nc.tensor.matmul(qA[:, 2:4, :], lhsT, xts[b][:, kc, 2:4, :],
                                         start=st_, stop=sp_)
                        nc.tensor.matmul(qB, lhsT, xts[b][:, kc, 4, :],
                                         start=st_, stop=sp_)
                    evac(q[:, mc, 0:4, :], qA)
                    evac(q[:, mc, 4, :], qB)
                qs[b] = q

            def emit_q0_split(b):
                """item-0 q in two passes. pass 1 (a0-1) is kc-OUTER with all
                8 mc psum groups open (uses the whole psum): matmuls start
                after just opt0+opt1+W0 and pace with the W chunk arrivals."""
                q = qp.tile([P, HC, NOPT, S], F16, tag="qq", name=f"q_{b}")
                grp = []
                for mc in range(4):
                    if mc % 2 == 0:
                        t = ps_big.tile([P, 4, S], FP32, tag="big",
                                        name=f"q0a_{mc}")
                        grp.append(t[:, 0:2, :])
                        grp.append(t[:, 2:4, :])
                for mc in range(4, 7):
                    t = ps_o.tile([P, 4, P], FP32, tag="o", name=f"q0a_{mc}")
                    grp.append(t.rearrange("p a i -> p (a i)")
                               .rearrange("p (a s) -> p a s", s=S))
                t = ps_sm.tile([P, 2, S], FP32, tag="sm", name="q0a_7")
                grp.append(t)
                for kc in range(HC):
                    for mc in range(HC):
                        nc.tensor.matmul(grp[mc],
                                         w_sb[:, kc, mc * P:(mc + 1) * P],
                                         xts[b][:, kc, 0:2, :],
                                         start=(kc == 0), stop=(kc == HC - 1))
                for mc in range(HC):
                    evac(q[:, mc, 0:2, :], grp[mc])
                for mc in range(HC):
                    qA = ps_big.tile([P, 4, S], FP32, tag="big",
                                     name=f"q0b_{mc}")
                    qB = ps_sm.tile([P, S], FP32, tag="sm", name=f"q0c_{mc}")
                    for kc in range(HC):
                        lhsT = w_sb[:, kc, mc * P:(mc + 1) * P]
                        st_, sp_ = (kc == 0), (kc == HC - 1)
                        nc.tensor.matmul(qA[:, 2:4, :], lhsT, xts[b][:, kc, 2:4, :],
                                         start=st_, stop=sp_)
                        nc.tensor.matmul(qB, lhsT, xts[b][:, kc, 4, :],
                                         start=st_, stop=sp_)
                    evac(q[:, mc, 2:4, :], qA[:, 2:4, :])
                    evac(q[:, mc, 4, :], qB)
                qs[b] = q

            def st_alloc(b):
                st = stp.tile([P, NOPT, SC, NOPT, S], F16, tag="st", name=f"st_{b}")
                for k in range(NOPT):
                    nc.gpsimd.memset(st[:, k, :, k, :], NEG)
                return st

            def emit_scores(b, st):
                """full-i scores: psum [P,4,S] groups per (k,jb)."""
                for k in range(NOPT):
                    for jb in range(SC):
                        pb = ps_big.tile([P, 4, S], FP32, tag="big",
                                         name=f"ps_{b}_{k}_{jb}")
                        first_in_bank = {}
                        last_in_bank = {}
                        for si, (c0, a0, na) in enumerate(SEGS[k]):
                            bank = c0 // 2
                            first_in_bank.setdefault(bank, si)
                            last_in_bank[bank] = si
                        for hc in range(HC):
                            lhsT = xts[b][:, hc, k, jb * P:(jb + 1) * P]
                            for si, (c0, a0, na) in enumerate(SEGS[k]):
                                bank = c0 // 2
                                nc.tensor.matmul(
                                    pb[:, c0:c0 + na, :], lhsT,
                                    qs[b][:, hc, a0:a0 + na, :],
                                    start=(hc == 0 and first_in_bank[bank] == si),
                                    stop=(hc == HC - 1 and last_in_bank[bank] == si))
                        if k > 0:
                            evac(st[:, k, jb, 0:k, :], pb[:, 0:k, :])
                        if k < NOPT - 1:
                            evac(st[:, k, jb, k + 1:NOPT, :], pb[:, k:4, :])

            def emit_scores_half(b, st, ih):
                """i-half scores for the tail item: 1-bank psum per (k,jb)."""
                isl = slice(ih * P, (ih + 1) * P)
                for k in range(NOPT):
                    for jb in range(SC):
                        # alternate the two psum pools: 5 groups in flight so
                        # a briefly blocked ACT evac queue can't stall the PE
                        if (k * SC + jb) % 2 == 0:
                            pb = ps_o.tile([P, 4, P], FP32, tag="o",
                                           name=f"ph_{b}_{k}_{jb}_{ih}")
                        else:
                            pb = ps_big.tile([P, 4, P], FP32, tag="big",
                                             name=f"ph_{b}_{k}_{jb}_{ih}")
                        nseg = len(SEGS[k])
                        for hc in range(HC):
                            lhsT = xts[b][:, hc, k, jb * P:(jb + 1) * P]
                            for si, (c0, a0, na) in enumerate(SEGS[k]):
                                nc.tensor.matmul(
                                    pb[:, c0:c0 + na, :], lhsT,
                                    qs[b][:, hc, a0:a0 + na, isl],
                                    start=(hc == 0 and si == 0),
                                    stop=(hc == HC - 1 and si == nseg - 1))
                        if k > 0:
                            evac(st[:, k, jb, 0:k, isl], pb[:, 0:k, :], eng="act")
                        if k < NOPT - 1:
                            evac(st[:, k, jb, k + 1:NOPT, isl], pb[:, k:4, :],
                                 eng="act")

            def emit_softmax(b, st, ws, cs=slice(0, S)):
                """softmax over k, batched over (jb, a, i in cs); jb-pipelined."""
                n = cs.stop - cs.start
                m = mzp.tile([P, SC, NOPT, S], F16, tag="mz", name=f"m_{b}_{cs.start}")
                z = mzp.tile([P, SC, NOPT, S], F16, tag="mz", name=f"z_{b}_{cs.start}")
                r = rp.tile([P, SC, NOPT, S], F16, tag="rr", name=f"r_{b}_{cs.start}")
                mc = m[:, :, :, cs]
                nc.vector.tensor_max(mc, st[:, 0, :, :, cs], st[:, 1, :, :, cs])
                for k in range(2, NOPT):
                    nc.vector.tensor_max(mc, mc, st[:, k, :, :, cs])
                for jb in range(SC):
                    stj = st[:, :, jb, :, cs]          # [P, 5k, 5a, n]
                    if BCAST:
                        mb = m[:, jb, :, cs].unsqueeze(1).broadcast_to(
                            (P, NOPT, NOPT, n))
                        nc.vector.tensor_sub(stj, stj, mb)
                    else:
                        for k in range(NOPT):
                            nc.vector.tensor_sub(st[:, k, jb, :, cs],
                                                 st[:, k, jb, :, cs],
                                                 m[:, jb, :, cs])
                    # exp in k-chunks: a single big exp (~5.7us HW) would
                    # block ACT-queued psum evacs and stall the PE
                    for k0, k1 in ((0, 2), (2, 4), (4, 5)):
                        nc.scalar.activation(out=st[:, k0:k1, jb, :, cs],
                                             in_=st[:, k0:k1, jb, :, cs],
                                             func=AF.Exp)
                    zj = z[:, jb, :, cs]
                    nc.vector.tensor_add(zj, st[:, 0, jb, :, cs],
                                         st[:, 1, jb, :, cs])
                    for k in range(2, NOPT):
                        nc.vector.tensor_add(zj, zj, st[:, k, jb, :, cs])
                    rj = r[:, jb, :, cs]
                    with nc.allow_low_precision(reason="1/z f16: w err ~5e-4"):
                        nc.vector.reciprocal(rj, zj)
                    if BCAST:
                        rb = rj.unsqueeze(1).broadcast_to((P, NOPT, NOPT, n))
                        nc.vector.tensor_mul(stj, stj, rb)
                    else:
                        for k in range(NOPT):
                            nc.vector.tensor_mul(st[:, k, jb, :, cs],
                                                 st[:, k, jb, :, cs],
                                                 r[:, jb, :, cs])
                    # ws[k] = sum_a w[k, a]; diagonal contributes 0.
                    # gpsimd (idle) for the hidden items; DVE for the tail
                    # item where ws is on the out(3) critical path.
                    we = nc.vector if cs.stop - cs.start < S else nc.gpsimd
                    we.tensor_add(st[:, :, jb, 0, cs], st[:, :, jb, 0, cs],
                                  st[:, :, jb, 1, cs])
                    we.tensor_add(st[:, :, jb, 2, cs], st[:, :, jb, 2, cs],
                                  st[:, :, jb, 3, cs])
                    we.tensor_add(st[:, :, jb, 0, cs], st[:, :, jb, 0, cs],
                                  st[:, :, jb, 2, cs])
                    we.tensor_add(ws[:, :, jb, cs], st[:, :, jb, 0, cs],
                                  st[:, :, jb, 4, cs])

            def emit_out(b, ws, ibs=(0, 1)):
                osb = op_.tile([P, SC, H], F16, tag="osb", name=f"osb_{b}")
                oap = out_d.ap()[b].rearrange("(sc p) h -> p sc h", p=P)
                for ib in ibs:
                    pos = [ps_o.tile([P, 512], FP32, tag="o",
                                     name=f"po_{b}_{ib}_{hb}") for hb in range(2)]
                    n = 0
                    for k in range(NOPT):
                        for jb in range(SC):
                            lhsT = ws[:, k, jb, ib * P:(ib + 1) * P]
                            st_, sp_ = (n == 0), (n == 2 * NOPT - 1)
                            for hb in range(2):
                                nc.tensor.matmul(pos[hb], lhsT,
                                                 nats[b][:, k, jb,
                                                         hb * 512:(hb + 1) * 512],
                                                 start=st_, stop=sp_)
                            n += 1
                    for hb in range(2):
                        evac(osb[:, ib, hb * 512:(hb + 1) * 512], pos[hb],
                             scale=0.5)
                    nc.scalar.dma_start(out=oap[:, ib, :], in_=osb[:, ib, :])

            # ---- schedule (PE order):
            # q0 s0 q1 s1 q2 o0 s2 q3 s3.h0 s3.h1 o1 o2 o3.ib0 o3.ib1
            load_xt(0, first=True)
            load_nat(0)
            load_xt(1)
            emit_q0_split(0)

            def ws_alloc(b):
                return wsp.tile([P, NOPT, SC, S], F16, tag="ws", name=f"ws_{b}")

            ws_all = [None] * BPC
            st0 = st_alloc(0)
            emit_scores(0, st0)
            load_nat(1)
            emit_q(1)
            ws_all[0] = ws_alloc(0)
            emit_softmax(0, st0, ws_all[0])
            st1 = st_alloc(1)
            emit_scores(1, st1)
            load_xt(2)
            emit_q(2)
            ws_all[1] = ws_alloc(1)
            emit_softmax(1, st1, ws_all[1])
            emit_out(0, ws_all[0])
            load_xt(3)
            st2 = st_alloc(2)
            emit_scores(2, st2)
            load_nat(2)
            emit_q(3)
            st3 = st_alloc(3)
            ws_all[3] = ws_alloc(3)
            emit_scores_half(3, st3, 0)
            # sm(2) emitted after s3.h0 so the h0 psum evacs (ACT) are not
            # queued behind sm(2)'s exp in the ACT FIFO
            ws_all[2] = ws_alloc(2)
            emit_softmax(2, st2, ws_all[2])
            emit_softmax(3, st3, ws_all[3], cs=slice(0, P))
            emit_scores_half(3, st3, 1)
            emit_softmax(3, st3, ws_all[3], cs=slice(P, S))
            emit_out(1, ws_all[1])
            load_nat(3)
            emit_out(2, ws_all[2])
            emit_out(3, ws_all[3])

    nc.compile()
    return nc


def _get_nc(reps: int = 1, cfg: dict | None = None):
    key = f"nc{reps}-{sorted((cfg or {}).items())}"
    if key not in _CACHE:
        _CACHE[key] = _build_bass(reps, cfg)
    return _CACHE[key]


def _prep(inputs):
    opts = [np.asarray(inputs[f"option{i + 1}"], dtype=np.float32)
            for i in range(NOPT)]
    nat = [np.ascontiguousarray(o.astype(np.float16)) for o in opts]
    xt = [np.ascontiguousarray(o.transpose(0, 2, 1).astype(np.float16))
          for o in opts]
    W = np.ascontiguousarray(np.asarray(inputs["W"], np.float32).astype(np.float16))
    return nat, xt, W


def kernel(**inputs) -> np.ndarray:
    from concourse.bass_utils import run_bass_kernel_spmd

    nc = _get_nc()
    nat, xt, W = _prep(inputs)

    in_maps = []
    for c in range(NCORES):
        m = {}
        for i in range(NOPT):
            m[f"xt{i + 1}"] = xt[i][c * BPC:(c + 1) * BPC]
            m[f"nat{i + 1}"] = nat[i][c * BPC:(c + 1) * BPC]
        m["W"] = W
        in_maps.append(m)

    res = run_bass_kernel_spmd(nc, in_maps, list(range(NCORES)))
    out = np.concatenate([res.results[c]["out"] for c in range(NCORES)], axis=0)
    return np.asarray(out, dtype=np.float32)


# revision 4
# speedup vs baseline: 1.3030x; 1.3030x over previous
"""ChoiceAttention Trainium2 kernel v2.

vs v1 (448 us HW):
  - scores land in ONE tile ST[P, k=5, jb=2, a=5, i=256] (diagonal a==k slots
    memset to -30000 by gpsimd): softmax batches elementwise ops over all a
    and k (k via stride-0 broadcast), cutting DVE instrs/item from ~85 to ~26.
  - softmax stages pipelined per jb (sub/exp/z/recip/mul/ws) so ACT exp
    overlaps DVE work.
  - item 3 (the tail) computes scores+softmax per i-half: softmax half 1
    hides under out(1)+out(2), out(3).ib0 needs only half 0.
  - dedicated 1-bank psum pool for out groups + item-3 score groups: out
    matmuls no longer wait on scores psum rotation.
  - W DMA split per-kc, interleaved into xt(0) option DMAs; nat deferred.
"""

import numpy as np

B, S, H = 32, 256, 1024
NCORES = 8
BPC = B // NCORES
P = 128
HC = H // P   # 8
SC = S // P   # 2
NOPT = 5

# scores psum layout per k: columns = a ascending, skipping a==k
SEGS = {
    0: [(0, 1, 2), (2, 3, 2)],
    1: [(0, 0, 1), (1, 2, 1), (2, 3, 2)],
    2: [(0, 0, 2), (2, 3, 2)],
    3: [(0, 0, 2), (2, 2, 1), (3, 4, 1)],
    4: [(0, 0, 2), (2, 2, 2)],
}

NEG = -30000.0

_CACHE: dict = {}


def _build_bass(reps: int = 1, cfg: dict | None = None):
    cfg = dict(cfg or {})
    BCAST = cfg.get("bcast", True)
    from contextlib import ExitStack, nullcontext

    import concourse.mybir as mybir
    import concourse.tile as tile
    from concourse import bacc

    FP32 = mybir.dt.float32
    F16 = mybir.dt.float16
    AF = mybir.ActivationFunctionType

    nc = bacc.Bacc(debug=False)

    xt_d = [nc.dram_tensor(f"xt{i + 1}", (BPC, H, S), F16, kind="ExternalInput")
            for i in range(NOPT)]
    nat_d = [nc.dram_tensor(f"nat{i + 1}", (BPC, S, H), F16, kind="ExternalInput")
             for i in range(NOPT)]
    w_d = nc.dram_tensor("W", (H, H), F16, kind="ExternalInput")
    out_d = nc.dram_tensor("out", (BPC, S, H), F16, kind="ExternalOutput")

    with ExitStack() as ctx:
        tc = ctx.enter_context(tile.TileContext(nc))
        const = ctx.enter_context(tc.tile_pool(name="const", bufs=1))
        xtp = ctx.enter_context(tc.tile_pool(name="xt", bufs=2))
        natp = ctx.enter_context(tc.tile_pool(name="nat", bufs=2))
        qp = ctx.enter_context(tc.tile_pool(name="qq", bufs=1))
        stp = ctx.enter_context(tc.tile_pool(name="st", bufs=2))
        mzp = ctx.enter_context(tc.tile_pool(name="mz", bufs=2))
        rp = ctx.enter_context(tc.tile_pool(name="rr", bufs=1))
        wsp = ctx.enter_context(tc.tile_pool(name="ws", bufs=3))
        op_ = ctx.enter_context(tc.tile_pool(name="osb", bufs=2))
        ps_big = ctx.enter_context(tc.tile_pool(name="ps_big", bufs=2, space="PSUM"))
        ps_sm = ctx.enter_context(tc.tile_pool(name="ps_sm", bufs=1, space="PSUM"))
        ps_o = ctx.enter_context(tc.tile_pool(name="ps_o", bufs=3, space="PSUM"))

        w_sb = const.tile([P, HC, H], F16)
        wap = w_d.ap().rearrange("(kc p) m -> p kc m", p=P)

        loop_cm = tc.For_i(0, reps, 1) if reps > 1 else nullcontext()
        with loop_cm:
            xts = [None] * BPC
            nats = [None] * BPC
            qs = [None] * BPC

            def xt_dma(b, kk):
                nc.sync.dma_start(
                    out=xts[b][:, :, kk, :],
                    in_=xt_d[kk].ap()[b].rearrange("(hc p) s -> p hc s", p=P))

            def load_xt(b, first=False):
                xts[b] = xtp.tile([P, HC, NOPT, S], F16, tag="xt", name=f"xt_{b}")
                if first:
                    # interleave: q(0) pass 1 starts after opt0.h0, opt1.h0, W0
                    def wc(kc):
                        nc.scalar.dma_start(out=w_sb[:, kc, :], in_=wap[:, kc, :])

                    def xt_half(kk, h):
                        hs = slice(h * 4, (h + 1) * 4)
                        nc.sync.dma_start(
                            out=xts[b][:, hs, kk, :],
                            in_=xt_d[kk].ap()[b].rearrange(
                                "(hc p) s -> p hc s", p=P)[:, hs, :])
                    xt_half(0, 0)
                    xt_half(1, 0)
                    wc(0)
                    wc(1)
                    xt_half(0, 1)
                    xt_half(1, 1)
                    wc(2)
                    wc(3)
                    xt_dma(b, 2)
                    wc(4)
                    wc(5)
                    xt_dma(b, 3)
                    for kc in range(6, HC):
                        wc(kc)
                    xt_dma(b, 4)
                else:
                    for kk in range(NOPT):
                        xt_dma(b, kk)

            def load_nat(b):
                nats[b] = natp.tile([P, NOPT, SC, H], F16, tag="nat",
                                    name=f"nat_{b}")
                for kk in range(NOPT):
                    nc.sync.dma_start(
                        out=nats[b][:, kk, :, :],
                        in_=nat_d[kk].ap()[b].rearrange("(sc p) h -> p sc h", p=P))

            def evac(out_ap, in_ap, eng="act", scale=None):
                # HW probe: psum->sbuf copies are cheap on ACT (~195ns for
                # [P,1024]f32) while DVE time is precious (softmax chain)
                if scale is None:
                    if eng == "act":
                        nc.scalar.copy(out=out_ap, in_=in_ap)
                    else:
                        nc.vector.tensor_copy(out=out_ap, in_=in_ap)
                else:
                    if eng == "act":
                        nc.scalar.activation(out=out_ap, in_=in_ap, func=AF.Copy,
                                             scale=scale)
                    else:
                        nc.vector.tensor_scalar_mul(out_ap, in_ap, scale)

            def emit_q(b):
                """Q(b) = q^T for all 5 options, W stationary, a-batched."""
                q = qp.tile([P, HC, NOPT, S], F16, tag="qq", name=f"q_{b}")
                for mc in range(HC):
                    qA = ps_big.tile([P, 4, S], FP32, tag="big", name=f"qA_{b}_{mc}")
                    qB = ps_sm.tile([P, S], FP32, tag="sm", name=f"qB_{b}_{mc}")
                    for kc in range(HC):
                        lhsT = w_sb[:, kc, mc * P:(mc + 1) * P]
                        st_, sp_ = (kc == 0), (kc == HC - 1)
                        nc.tensor.matmul(qA[:, 0:2, :], lhsT, xts[b][:, kc, 0:2, :],
                                         start=st_, stop=sp_)
                        nc.tensor.matmul(qA[:, 2:4, :], lhsT, xts[b][:, kc, 2:4, :],
                                         start=st_, stop=sp_)
                        nc.tensor.matmul(qB, lhsT, xts[b][:, kc, 4, :],
                                         start=st_, stop=sp_)
                    evac(q[:, mc, 0:4, :], qA)
                    evac(q[:, mc, 4, :], qB)
                qs[b] = q

            def emit_q0_split(b):
                """item-0 q in two passes. pass 1 (a0-1) is kc-OUTER with all
                8 mc psum groups open (uses the whole psum): matmuls start
                after just opt0+opt1+W0 and pace with the W chunk arrivals."""
                q = qp.tile([P, HC, NOPT, S], F16, tag="qq", name=f"q_{b}")
                grp = []
                for mc in range(4):
                    if mc % 2 == 0:
                        t = ps_big.tile([P, 4, S], FP32, tag="big",
                                        name=f"q0a_{mc}")
                        grp.append(t[:, 0:2, :])
                        grp.append(t[:, 2:4, :])
                for mc in range(4, 7):
                    t = ps_o.tile([P, 4, P], FP32, tag="o", name=f"q0a_{mc}")
                    grp.append(t.rearrange("p a i -> p (a i)")
                               .rearrange("p (a s) -> p a s", s=S))
                t = ps_sm.tile([P, 2, S], FP32, tag="sm", name="q0a_7")
                grp.append(t)
                for kc in range(HC):
                    for mc in range(HC):
                        nc.tensor.matmul(grp[mc],
                                         w_sb[:, kc, mc * P:(mc + 1) * P],
                                         xts[b][:, kc, 0:2, :],
                                         start=(kc == 0), stop=(kc == HC - 1))
                for mc in range(HC):
                    evac(q[:, mc, 0:2, :], grp[mc])
                for mc in range(HC):
                    qA = ps_big.tile([P, 4, S], FP32, tag="big",
                                     name=f"q0b_{mc}")
                    qB = ps_sm.tile([P, S], FP32, tag="sm", name=f"q0c_{mc}")
                    for kc in range(HC):
                        lhsT = w_sb[:, kc, mc * P:(mc + 1) * P]
                        st_, sp_ = (kc == 0), (kc == HC - 1)
                        nc.tensor.matmul(qA[:, 2:4, :], lhsT, xts[b][:, kc, 2:4, :],
                                         start=st_, stop=sp_)
                        nc.tensor.matmul(qB, lhsT, xts[b][:, kc, 4, :],
                                         start=st_, stop=sp_)
                    evac(q[:, mc, 2:4, :], qA[:, 2:4, :])
                    evac(q[:, mc, 4, :], qB)
                qs[b] = q

            def st_alloc(b):
                st = stp.tile([P, NOPT, SC, NOPT, S], F16, tag="st", name=f"st_{b}")
                for k in range(NOPT):
                    nc.gpsimd.memset(st[:, k, :, k, :], NEG)
                return st

            def emit_scores(b, st):
                """full-i scores: psum [P,4,S] groups per (k,jb)."""
                for k in range(NOPT):
                    for jb in range(SC):
                        pb = ps_big.tile([P, 4, S], FP32, tag="big",
                                         name=f"ps_{b}_{k}_{jb}")
                        first_in_bank = {}
                        last_in_bank = {}
                        for si, (c0, a0, na) in enumerate(SEGS[k]):
                            bank = c0 // 2
                            first_in_bank.setdefault(bank, si)
                            last_in_bank[bank] = si
                        for hc in range(HC):
                            lhsT = xts[b][:, hc, k, jb * P:(jb + 1) * P]
                            for si, (c0, a0, na) in enumerate(SEGS[k]):
                                bank = c0 // 2
                                nc.tensor.matmul(
                                    pb[:, c0:c0 + na, :], lhsT,
                                    qs[b][:, hc, a0:a0 + na, :],
                                    start=(hc == 0 and first_in_bank[bank] == si),
                                    stop=(hc == HC - 1 and last_in_bank[bank] == si))
                        if k > 0:
                            evac(st[:, k, jb, 0:k, :], pb[:, 0:k, :])
                        if k < NOPT - 1:
                            evac(st[:, k, jb, k + 1:NOPT, :], pb[:, k:4, :])

            def emit_scores_half(b, st, ih):
                """i-half scores for the tail item: 1-bank psum per (k,jb)."""
                isl = slice(ih * P, (ih + 1) * P)
                for k in range(NOPT):
                    for jb in range(SC):
                        # alternate the two psum pools: 5 groups in flight so
                        # a briefly blocked ACT evac queue can't stall the PE
                        if (k * SC + jb) % 2 == 0:
                            pb = ps_o.tile([P, 4, P], FP32, tag="o",
                                           name=f"ph_{b}_{k}_{jb}_{ih}")
                        else:
                            pb = ps_big.tile([P, 4, P], FP32, tag="big",
                                             name=f"ph_{b}_{k}_{jb}_{ih}")
                        nseg = len(SEGS[k])
                        for hc in range(HC):
                            lhsT = xts[b][:, hc, k, jb * P:(jb + 1) * P]
                            for si, (c0, a0, na) in enumerate(SEGS[k]):
                                nc.tensor.matmul(
                                    pb[:, c0:c0 + na, :], lhsT,
                                    qs[b][:, hc, a0:a0 + na, isl],
                                    start=(hc == 0 and si == 0),
                                    stop=(hc == HC - 1 and si == nseg - 1))
                        if k > 0:
                            evac(st[:, k, jb, 0:k, isl], pb[:, 0:k, :], eng="act")
                        if k < NOPT - 1:
                            evac(st[:, k, jb, k + 1:NOPT, isl], pb[:, k:4, :],
                                 eng="act")

            def emit_softmax(b, st, ws, cs=slice(0, S)):
                """softmax over k, batched over (jb, a, i in cs); jb-pipelined."""
                n = cs.stop - cs.start
                m = mzp.tile([P, SC, NOPT, S], F16, tag="mz", name=f"m_{b}_{cs.start}")
                z = mzp.tile([P, SC, NOPT, S], F16, tag="mz", name=f"z_{b}_{cs.start}")
                r = rp.tile([P, SC, NOPT, S], F16, tag="rr", name=f"r_{b}_{cs.start}")
                mc = m[:, :, :, cs]
                nc.vector.tensor_max(mc, st[:, 0, :, :, cs], st[:, 1, :, :, cs])
                for k in range(2, NOPT):
                    nc.vector.tensor_max(mc, mc, st[:, k, :, :, cs])
                for jb in range(SC):
                    stj = st[:, :, jb, :, cs]          # [P, 5k, 5a, n]
                    if BCAST:
                        mb = m[:, jb, :, cs].unsqueeze(1).broadcast_to(
                            (P, NOPT, NOPT, n))
                        nc.vector.tensor_sub(stj, stj, mb)
                    else:
                        for k in range(NOPT):
                            nc.vector.tensor_sub(st[:, k, jb, :, cs],
                                                 st[:, k, jb, :, cs],
                                                 m[:, jb, :, cs])
                    # exp in k-chunks: a single big exp (~5.7us HW) would
                    # block ACT-queued psum evacs and stall the PE
                    for k0, k1 in ((0, 2), (2, 4), (4, 5)):
                        nc.scalar.activation(out=st[:, k0:k1, jb, :, cs],
                                             in_=st[:, k0:k1, jb, :, cs],
                                             func=AF.Exp)
                    zj = z[:, jb, :, cs]
                    nc.vector.tensor_add(zj, st[:, 0, jb, :, cs],
                                         st[:, 1, jb, :, cs])
                    for k in range(2, NOPT):
                        nc.vector.tensor_add(zj, zj, st[:, k, jb, :, cs])
                    rj = r[:, jb, :, cs]
                    with nc.allow_low_precision(reason="1/z f16: w err ~5e-4"):
                        nc.vector.reciprocal(rj, zj)
                    if BCAST:
                        rb = rj.unsqueeze(1).broadcast_to((P, NOPT, NOPT, n))
                        nc.vector.tensor_mul(stj, stj, rb)
                    else:
                        for k in range(NOPT):
                            nc.vector.tensor_mul(st[:, k, jb, :, cs],
                                                 st[:, k, jb, :, cs],
                                                 r[:, jb, :, cs])
                    # ws[k] = sum_a w[k, a]; diagonal contributes 0.
                    # always DVE: ws gates out(b), and chain latency (not DVE
                    # occupancy) is what binds; gpsimd is slower per element.
                    we = nc.vector
                    we.tensor_add(st[:, :, jb, 0, cs], st[:, :, jb, 0, cs],
                                  st[:, :, jb, 1, cs])
                    we.tensor_add(st[:, :, jb, 2, cs], st[:, :, jb, 2, cs],
                                  st[:, :, jb, 3, cs])
                    we.tensor_add(st[:, :, jb, 0, cs], st[:, :, jb, 0, cs],
                                  st[:, :, jb, 2, cs])
                    we.tensor_add(ws[:, :, jb, cs], st[:, :, jb, 0, cs],
                                  st[:, :, jb, 4, cs])

            def emit_out(b, ws, ibs=(0, 1)):
                osb = op_.tile([P, SC, H], F16, tag="osb", name=f"osb_{b}")
                oap = out_d.ap()[b].rearrange("(sc p) h -> p sc h", p=P)
                for ib in ibs:
                    pos = [ps_o.tile([P, 512], FP32, tag="o",
                                     name=f"po_{b}_{ib}_{hb}") for hb in range(2)]
                    n = 0
                    for k in range(NOPT):
                        for jb in range(SC):
                            lhsT = ws[:, k, jb, ib * P:(ib + 1) * P]
                            st_, sp_ = (n == 0), (n == 2 * NOPT - 1)
                            for hb in range(2):
                                nc.tensor.matmul(pos[hb], lhsT,
                                                 nats[b][:, k, jb,
                                                         hb * 512:(hb + 1) * 512],
                                                 start=st_, stop=sp_)
                            n += 1
                    for hb in range(2):
                        evac(osb[:, ib, hb * 512:(hb + 1) * 512], pos[hb],
                             scale=0.5)
                    nc.scalar.dma_start(out=oap[:, ib, :], in_=osb[:, ib, :])

            # ---- schedule (PE order):
            # q0 s0 q1 s1 q2 o0 s2 q3 s3.h0 s3.h1 o1 o2 o3.ib0 o3.ib1
            load_xt(0, first=True)
            load_nat(0)
            load_xt(1)
            emit_q0_split(0)

            def ws_alloc(b):
                return wsp.tile([P, NOPT, SC, S], F16, tag="ws", name=f"ws_{b}")

            ws_all = [None] * BPC
            st0 = st_alloc(0)
            emit_scores(0, st0)
            load_nat(1)
            emit_q(1)
            ws_all[0] = ws_alloc(0)
            emit_softmax(0, st0, ws_all[0])
            st1 = st_alloc(1)
            emit_scores(1, st1)
            load_xt(2)
            emit_q(2)
            ws_all[1] = ws_alloc(1)
            emit_softmax(1, st1, ws_all[1])
            emit_out(0, ws_all[0])
            load_xt(3)
            st2 = st_alloc(2)
            emit_scores(2, st2)
            load_nat(2)
            emit_q(3)
            # sm(2) right after s2: its HW chain (~40us, 2x the sim model)
            # must clear before out(2); the chunked exps keep the ACT FIFO
            # from blocking the s3 psum evacs behind them
            ws_all[2] = ws_alloc(2)
            emit_softmax(2, st2, ws_all[2])
            st3 = st_alloc(3)
            ws_all[3] = ws_alloc(3)
            emit_scores_half(3, st3, 0)
            emit_softmax(3, st3, ws_all[3], cs=slice(0, P))
            emit_scores_half(3, st3, 1)
            emit_softmax(3, st3, ws_all[3], cs=slice(P, S))
            emit_out(1, ws_all[1])
            load_nat(3)
            emit_out(2, ws_all[2])
            emit_out(3, ws_all[3])

    nc.compile()
    return nc


def _get_nc(reps: int = 1, cfg: dict | None = None):
    key = f"nc{reps}-{sorted((cfg or {}).items())}"
    if key not in _CACHE:
        _CACHE[key] = _build_bass(reps, cfg)
    return _CACHE[key]


def _prep(inputs):
    opts = [np.asarray(inputs[f"option{i + 1}"], dtype=np.float32)
            for i in range(NOPT)]
    nat = [np.ascontiguousarray(o.astype(np.float16)) for o in opts]
    xt = [np.ascontiguousarray(o.transpose(0, 2, 1).astype(np.float16))
          for o in opts]
    W = np.ascontiguousarray(np.asarray(inputs["W"], np.float32).astype(np.float16))
    return nat, xt, W


def kernel(**inputs) -> np.ndarray:
    from concourse.bass_utils import run_bass_kernel_spmd

    nc = _get_nc()
    nat, xt, W = _prep(inputs)

    in_maps = []
    for c in range(NCORES):
        m = {}
        for i in range(NOPT):
            m[f"xt{i + 1}"] = xt[i][c * BPC:(c + 1) * BPC]
            m[f"nat{i + 1}"] = nat[i][c * BPC:(c + 1) * BPC]
        m["W"] = W
        in_maps.append(m)

    res = run_bass_kernel_spmd(nc, in_maps, list(range(NCORES)))
    out = np.concatenate([res.results[c]["out"] for c in range(NCORES)], axis=0)
    return np.asarray(out, dtype=np.float32)
